# revision 17
# baseline (speedup 1.0000x reference)
"""Trainium2 Bass/Tile kernel for nn_Attention_50242527428847.

Computation (per batch element b, one NeuronCore each):
    dec[t,e]   = sum_h decoder_states[t,b,h] * W[e,h]            (projection)
    p[t,s,e]   = exp(dec[t,e] * encoder_states[s,b,e])
    denom[t,e] = sum_s p[t,s,e]
    wsum[t,s]  = sum_e p[t,s,e] / denom[t,e]
    out[t,b,d] = sum_s wsum[t,s] * encoder_inputs[s,b,d]

Design (v2, mixed fp16/bf16, ce-major pipeline):
  - All elementwise tensors are 2-byte: scores fp16 (good absolute precision
    in the exponent), p bf16 (needs e^{+-17} range; fp16 would overflow /
    flush), weights/activations for matmuls fp16/bf16.
  - Layout p[e_local(128 part), t, s] one E-chunk (ce) at a time; the 4
    ce-chunks stream through a double-buffered 32KB tile instead of holding
    the whole 128KB tensor.
  - Engine split per ce (balanced to ~15us each):
      PE:   projection (fp16, 1cyc/col), per-t wsum matmuls (N=1,
            accumulated over ce in a PSUM group per t-column), final
            out = wsumT.T @ enc_in (fp16, N=256).
      DVE:  part of the scores broadcast-mult (1x - broadcast APs disable
            the 2x fp16 mode), denominator tree-add levels 1..4 (bf16 2x),
            reciprocal, small copies.
      Pool: rest of the scores mult via scalar_tensor_tensor (1.36ns/elem
            vs 1.96 for plain tensor_tensor), tree tail levels 5..7.
      ACT:  all exps (fp16 in -> bf16 out), big-free instructions; a few
            t-columns use the fused exp(enc*scale=dec) form to soak ACT
            slack when the sim shows DVE/Pool as critical.
  - No max-subtraction: |scores| <= ~17 for N(0,1)-scale inputs, inside
    bf16/fp32 exp range.
"""

import numpy as np
from contextlib import ExitStack

import concourse.bass as bass
import concourse.bacc as bacc
import concourse.tile as tile
from concourse import mybir
from concourse.bass_utils import run_bass_kernel_spmd

TD, TE, B = 128, 128, 8
E, H, D = 512, 1024, 256
P = 128
CE = E // P          # 4 e-chunks
CH = H // P          # 8 h-chunks
TB = 16              # t-block for the mult stage
NBLK = TD // TB      # 8 blocks per ce

_F32 = mybir.dt.float32
_F16 = mybir.dt.float16
_BF16 = mybir.dt.bfloat16
_CACHE = {}

# mult engine assignment per (ce, blk): 'P' Pool, 'V' DVE, 'A' ACT-fused
_SPLIT = ["PPPPVVVA", "PPPVVVVA", "PPPPVVVA", "PPPVVVVA"]
MULT_ENGINE = {}
for _ce in range(CE):
    for _blk in range(NBLK):
        MULT_ENGINE[(_ce, _blk)] = _SPLIT[_ce][_blk]


def _bcast(ap_in, new_dims):
    """Rebuild an AP with explicit free dims (list of [stride, count])."""
    return bass.AP(tensor=ap_in.tensor, offset=ap_in.offset,
                   ap=[ap_in.ap[0]] + new_dims)


def _kernel_body(ctx, tc, out_ap, wt_ap, dt_ap, et_ap, ei_ap):
    nc = tc.nc
    AF = mybir.ActivationFunctionType

    singles = ctx.enter_context(tc.tile_pool(name="singles", bufs=1))
    sc_pool = ctx.enter_context(tc.tile_pool(name="sc", bufs=2))
    p_pool = ctx.enter_context(tc.tile_pool(name="p", bufs=2))
    tr_pool = ctx.enter_context(tc.tile_pool(name="tr", bufs=2))
    psum_pool = ctx.enter_context(tc.tile_pool(name="psum", bufs=2, space="PSUM"))
    psum_w = ctx.enter_context(tc.tile_pool(name="psum_w", bufs=1, space="PSUM"))
    psum_o = ctx.enter_context(tc.tile_pool(name="psum_o", bufs=1, space="PSUM"))

    # ---- input DMAs (fp16 data, host-transposed), ce0's chain first,
    # split across both HW-DGE rings (SP=sync + ACT=scalar)
    dt_sb = singles.tile([P, CH, TD], _F16)      # decoder^T [h_local, hc, t]
    dt_r = dt_ap.rearrange("(c p) t -> p c t", p=P)
    nc.sync.dma_start(out=dt_sb[:], in_=dt_r[:])
    wt_sb = singles.tile([P, CH, E], _F16)       # W^T [h_local, hc, e]
    wt_r = wt_ap.rearrange("(c p) e -> p c e", p=P)
    et_sb = singles.tile([P, CE, TE], _F16)      # enc^T [e_local, ce, s]
    et_r = et_ap.rearrange("(c p) s -> p c s", p=P)
    nc.scalar.dma_start(out=wt_sb[:, :, 0:P], in_=wt_r[:, :, 0:P])
    nc.scalar.dma_start(out=et_sb[:, 0, :], in_=et_r[:, 0, :])
    for ce in range(1, CE):
        eng = nc.sync if ce % 2 == 1 else nc.scalar
        eng.dma_start(out=wt_sb[:, :, ce * P:(ce + 1) * P],
                      in_=wt_r[:, :, ce * P:(ce + 1) * P])
        eng.dma_start(out=et_sb[:, ce, :], in_=et_r[:, ce, :])
    ei_sb = singles.tile([P, D], _F16)           # enc_in [s, d]
    nc.scalar.dma_start(out=ei_sb[:], in_=ei_ap)

    # ---- projection: dec[e_local, ce, t] = sum_h W^T[h, e] * D^T[h, t]
    dec16 = singles.tile([P, CE, TD], _F16)
    dec32 = singles.tile([P, CE, TD], _F32)
    for ce in range(CE):
        dps = psum_pool.tile([P, TD], _F32, name="dps")
        for c in range(CH):
            nc.tensor.matmul(
                dps[:],
                lhsT=wt_sb[:, c, ce * P:(ce + 1) * P],
                rhs=dt_sb[:, c, :],
                start=(c == 0),
                stop=(c == CH - 1),
            )
        nc.vector.tensor_copy(dec16[:, ce, :], dps[:])
        nc.vector.tensor_copy(dec32[:, ce, :], dps[:])

    den16 = singles.tile([P, CE, TD], _BF16)
    den32 = singles.tile([P, CE, TD], _F32)
    r32 = singles.tile([P, CE, TD], _F32)
    r16 = singles.tile([P, CE, TD], _BF16)

    # one wsum PSUM tile per ce (summed at the end; interleaved PSUM
    # accumulation groups across the ce loop lose contributions)
    wps_all = [psum_w.tile([P, TD], _F32, name=f"wps{c}") for c in range(CE)]

    sc_ts = {}
    p_ts = {}

    def emit_mult_exp(ce):
        sc_t = sc_pool.tile([P, TD, TE], _F16, name="sc")
        p_t = p_pool.tile([P, TD, TE], _BF16, name="p")
        sc_ts[ce], p_ts[ce] = sc_t, p_t
        eslice = et_sb[:, ce, :]
        enc_b = _bcast(eslice, [[0, TB], eslice.ap[1]])
        # scores (broadcast mult) on V/P blocks, interleaved with ACT exp
        # per two blocks so the exp pipeline starts early
        runs = []
        for blk in range(NBLK):
            t0 = blk * TB
            eng = MULT_ENGINE[(ce, blk)]
            dslice = dec16[:, ce, t0:t0 + TB]
            dec_b = _bcast(dslice, [dslice.ap[1], [0, TE]])
            if eng == 'P':
                nc.gpsimd.tensor_mul(sc_t[:, t0:t0 + TB, :], enc_b, dec_b)
            elif eng == 'V':
                nc.vector.tensor_mul(sc_t[:, t0:t0 + TB, :], enc_b, dec_b)
            else:  # 'A': fused exp(enc*dec) straight into p
                for tl in range(TB):
                    t = t0 + tl
                    nc.scalar.activation(
                        out=p_t[:, t, :], in_=et_sb[:, ce, :], func=AF.Exp,
                        scale=dec32[:, ce, t:t + 1])
                continue
            if runs and runs[-1][1] == blk and runs[-1][1] - runs[-1][0] < 2:
                runs[-1][1] = blk + 1
            else:
                runs.append([blk, blk + 1])
            if runs[-1][1] - runs[-1][0] == 2:
                b0, b1 = runs[-1]
                nc.scalar.activation(out=p_t[:, b0 * TB:b1 * TB, :],
                                     in_=sc_t[:, b0 * TB:b1 * TB, :],
                                     func=AF.Exp)
        for b0, b1 in runs:
            if b1 - b0 < 2:
                nc.scalar.activation(out=p_t[:, b0 * TB:b1 * TB, :],
                                     in_=sc_t[:, b0 * TB:b1 * TB, :],
                                     func=AF.Exp)

    def emit_tree_post(ce):
        # denominators: tree adds over s (levels 1-4 DVE bf16 2x, tail on
        # Pool), per t-64 halves
        p_t = p_ts[ce]
        tr_t = tr_pool.tile([P, TD, TE // 2], _BF16, name="tr")
        for g in range(2):
            sl = slice(g * 64, g * 64 + 64)
            nc.vector.tensor_add(tr_t[:, sl, 0:64], p_t[:, sl, 0:64],
                                 p_t[:, sl, 64:128])
            nc.vector.tensor_add(tr_t[:, sl, 0:32], tr_t[:, sl, 0:32],
                                 tr_t[:, sl, 32:64])
            nc.vector.tensor_add(tr_t[:, sl, 0:16], tr_t[:, sl, 0:16],
                                 tr_t[:, sl, 16:32])
            nc.vector.tensor_add(tr_t[:, sl, 0:8], tr_t[:, sl, 0:8],
                                 tr_t[:, sl, 8:16])
            nc.gpsimd.tensor_add(tr_t[:, sl, 0:4], tr_t[:, sl, 0:4],
                                 tr_t[:, sl, 4:8])
            nc.gpsimd.tensor_add(tr_t[:, sl, 0:2], tr_t[:, sl, 0:2],
                                 tr_t[:, sl, 2:4])
            nc.gpsimd.tensor_add(den16[:, ce, sl], tr_t[:, sl, 0:1],
                                 tr_t[:, sl, 1:2])
        # r = 1/denom (fp32), then bf16 copy for the matmul rhs
        nc.vector.tensor_copy(den32[:, ce, :], den16[:, ce, :])
        nc.vector.reciprocal_approx_fast(r32[:, ce, :], den32[:, ce, :])
        nc.vector.tensor_copy(r16[:, ce, :], r32[:, ce, :])

    def emit_wsum(ce, half):
        # wsum_T[s, t] += p[e, t, :].T @ r[e, t] (N=1 matmuls, ~free)
        p_t = p_ts[ce]
        for t in range(half * 64, half * 64 + 64):
            nc.tensor.matmul(
                wps_all[ce][:, t:t + 1],
                lhsT=p_t[:, t, :],
                rhs=r16[:, ce, t:t + 1],
                start=True,
                stop=True,
            )

    wsumT = singles.tile([P, TD], _F16)
    ws_a = singles.tile([P, TD], _F32)
    out_sb = singles.tile([P, D], _F32)
    ops = psum_o.tile([P, D], _F32)

    def emit_final(half):
        # out[t, d] = wsum_T.T @ enc_in, fp16, one t-64 chunk
        t0 = half * 64
        sl = slice(t0, t0 + 64)
        nc.vector.tensor_copy(ws_a[:, sl], wps_all[0][:, sl])
        nc.vector.tensor_add(ws_a[:, sl], ws_a[:, sl], wps_all[1][:, sl])
        nc.vector.tensor_add(ws_a[:, sl], ws_a[:, sl], wps_all[2][:, sl])
        nc.vector.tensor_add(ws_a[:, sl], ws_a[:, sl], wps_all[3][:, sl])
        nc.vector.tensor_copy(wsumT[:, sl], ws_a[:, sl])
        nc.tensor.matmul(ops[t0:t0 + 64, :], lhsT=wsumT[:, sl],
                         rhs=ei_sb[:], start=True, stop=True,
                         tile_position=(0, t0))
        nc.vector.tensor_copy(out_sb[t0:t0 + 64, :], ops[t0:t0 + 64, :])
        nc.sync.dma_start(out=out_ap[t0:t0 + 64, :], in_=out_sb[t0:t0 + 64, :])

    # ---- software-pipelined emission: ce+2's mult/exp precede ce+1's tree
    # in each engine's program order so engines never head-of-line block
    emit_mult_exp(0)
    emit_mult_exp(1)
    for ce in range(CE):
        emit_tree_post(ce)
        emit_wsum(ce, 0)
        if ce == CE - 1:
            emit_final(0)
        emit_wsum(ce, 1)
        if ce == CE - 1:
            emit_final(1)
        if ce + 2 < CE:
            emit_mult_exp(ce + 2)


def build_program():
    if "nc" in _CACHE:
        return _CACHE["nc"]
    nc = bacc.Bacc("TRN2", target_bir_lowering=False, debug=False, num_devices=B)
    wt = nc.dram_tensor("wt", [H, E], _F16, kind="ExternalInput").ap()
    dt = nc.dram_tensor("dt", [H, TD], _F16, kind="ExternalInput").ap()
    et = nc.dram_tensor("et", [E, TE], _F16, kind="ExternalInput").ap()
    ei = nc.dram_tensor("ei", [TE, D], _F16, kind="ExternalInput").ap()
    out = nc.dram_tensor("out", [TD, D], _F32, kind="ExternalOutput").ap()
    with tile.TileContext(nc) as tc:
        with ExitStack() as ctx:
            _kernel_body(ctx, tc, out, wt, dt, et, ei)
    nc.compile()
    _CACHE["nc"] = nc
    return nc


def make_in_maps(encoder_inputs, encoder_states, decoder_states, W):
    wt_np = np.ascontiguousarray(W.T).astype(np.float16)          # (H, E)
    in_maps = []
    for b in range(B):
        in_maps.append({
            "wt": wt_np,
            "dt": np.ascontiguousarray(decoder_states[:, b, :].T).astype(np.float16),
            "et": np.ascontiguousarray(encoder_states[:, b, :].T).astype(np.float16),
            "ei": np.ascontiguousarray(encoder_inputs[:, b, :]).astype(np.float16),
        })
    return in_maps


def run_on_hw(in_maps, **kwargs):
    nc = build_program()
    return run_bass_kernel_spmd(nc, in_maps, list(range(B)), **kwargs)


def kernel(**inputs):
    encoder_inputs = np.asarray(inputs["encoder_inputs"], dtype=np.float32)
    encoder_states = np.asarray(inputs["encoder_states"], dtype=np.float32)
    decoder_states = np.asarray(inputs["decoder_states"], dtype=np.float32)
    W = np.asarray(inputs["W"], dtype=np.float32)
    in_maps = make_in_maps(encoder_inputs, encoder_states, decoder_states, W)
    res = run_on_hw(in_maps)
    out = np.stack([res.results[b]["out"] for b in range(B)], axis=1)
    return np.ascontiguousarray(out.astype(np.float32))
